# revision 22
# baseline (speedup 1.0000x reference)
"""Multi-head attention (QKV proj + RoPE + softmax attention) on 8 Trainium2
NeuronCores, tensor-parallel over heads (2 heads per core).

v2: bf16 operand pipeline (same PE matmul rate as f32r, half the DMA/SBUF,
2-4x DVE), head-ping-pong PSUM in the projection phase, 4-way column-tiled
denominator matmuls, 1024-wide exp straight from PSUM, Vector-engine copies.

Contract: kernel(**inputs) takes the FULL unsharded inputs and returns the
FULL [B, S, H] float32 output.
"""

from contextlib import ExitStack

import numpy as np

B, S, H = 2, 2048, 2048
NH, D = 16, 128
ROPE_BASE = 10000.0
NCORES = 8
HPC = NH // NCORES          # heads per core
CH = HPC * D                # output channels per core
BS = B * S                  # flattened tokens
KT = H // 128               # contraction k-tiles
NCH = BS // 512             # 512-wide token chunks
SKT = S // 128              # score k-tiles per sequence
SQC = S // 512              # query chunks per sequence

LAST_RESULT = None          # BassKernelResults of the most recent run (for test.py)


def _build_nc(with_bias):
    import concourse.mybir as mybir
    import concourse.tile as tile
    from concourse import bacc

    F32 = mybir.dt.float32
    F32R = mybir.dt.float32r
    BF16 = mybir.dt.bfloat16
    AF = mybir.ActivationFunctionType
    ALU = mybir.AluOpType
    ISCALE = float(1.0 / np.sqrt(D))

    nc = bacc.Bacc("TRN2", debug=False, enable_partition_id=False)

    hsT_d = nc.dram_tensor("hsT", [H, BS], BF16, kind="ExternalInput").ap()
    wT_d = {
        p: nc.dram_tensor(f"w{p}T", [H, CH], BF16, kind="ExternalInput").ap()
        for p in "qkv"
    }
    b_d = {
        p: nc.dram_tensor(f"b{p}", [1, CH], F32R, kind="ExternalInput").ap()
        for p in "qkv"
    }
    cos_d = nc.dram_tensor("cosT", [D // 2, S], F32, kind="ExternalInput").ap()
    sin_d = nc.dram_tensor("sinT", [D // 2, S], F32, kind="ExternalInput").ap()
    out_d = nc.dram_tensor("out", [BS, CH], F32, kind="ExternalOutput").ap()

    with tile.TileContext(nc) as tc, ExitStack() as ctx:
        # ---- persistent state (lives across both phases) ----
        persist = ctx.enter_context(tc.tile_pool(name="persist", bufs=1))
        qT = [persist.tile([128, BS], BF16, tag=f"qT{m}", name=f"qT{m}") for m in range(HPC)]
        kTt = [persist.tile([128, BS], BF16, tag=f"kT{m}", name=f"kT{m}") for m in range(HPC)]
        vN = [persist.tile([128, BS // 128, D], BF16, tag=f"v{m}", name=f"vn{m}") for m in range(HPC)]

        consts = ctx.enter_context(tc.tile_pool(name="consts", bufs=1))
        ones_c = consts.tile([128, 32], BF16, tag="ones_c")
        nc.vector.memset(ones_c, 1.0)
        if with_bias:
            ones_row = consts.tile([1, 512], F32, tag="ones_row")
            nc.vector.memset(ones_row, 1.0)
            b_sb = {}
            for p in "qkv":
                b_sb[p] = consts.tile([1, CH], F32R, tag=f"b{p}", name=f"b{p}sb")
                nc.sync.dma_start(b_sb[p], b_d[p])

        # ================= Phase 1: QKV projections + RoPE =================
        with (
            tc.tile_pool(name="wpool", bufs=1) as wpool,
            tc.tile_pool(name="tabs", bufs=1) as tabs,
            tc.tile_pool(name="hstp", bufs=4) as hstp,
            tc.tile_pool(name="p1ps", bufs=2, space="PSUM") as p1ps,
            tc.tile_pool(name="ropet", bufs=3) as ropet,
        ):
            w_sb = {}
            for p in "qkv":
                w_sb[p] = wpool.tile([128, KT, CH], BF16, tag=f"w{p}", name=f"w{p}sb")
            w_r = {p: wT_d[p].rearrange("(k p) c -> p k c", p=128) for p in "qkv"}
            hsT_r = hsT_d.rearrange("(k p) t -> p k t", p=128)
            cos_sb = tabs.tile([D, S], F32, tag="cos")
            sin_sb = tabs.tile([D, S], F32, tag="sin")

            # DMA issue order tuned for a fast start: wq halves, first hs
            # chunk, then the rest.
            hs_tiles = {}
            hs_tiles[0] = hstp.tile([128, KT, 512], BF16, tag="hs", name="hs0")
            # wq pieces dispatch on the scalar DGE queue, hs pieces on sync:
            # the two startup streams issue in parallel instead of serially
            for lo, hi in ((0, 1), (1, 2), (2, 4), (4, 8), (8, 12), (12, 16)):
                hh = slice(lo, hi)
                nc.scalar.dma_start(w_sb["q"][:, hh, :], w_r["q"][:, hh, :])
                nc.sync.dma_start(hs_tiles[0][:, hh, :], hsT_r[:, hh, 0:512])
            for p in "kv":
                for h in range(2):
                    nc.sync.dma_start(w_sb[p][:, h * 8:(h + 1) * 8, :],
                                      w_r[p][:, h * 8:(h + 1) * 8, :])
            # rope tables have duplicated 64-row halves; ship only the
            # distinct rows (1MB not 2MB in the DMA-bound front, scalar
            # queue), mirror/negate on-chip once on the idle DVE.
            nc.scalar.dma_start(cos_sb[0:64, :], cos_d)
            nc.scalar.dma_start(sin_sb[0:64, :], sin_d)
            nc.vector.tensor_copy(cos_sb[64:128, :], cos_sb[0:64, :])
            nc.vector.tensor_scalar_mul(sin_sb[64:128, :], sin_sb[0:64, :], -1.0)

            for n in range(NCH):
                tok = slice(n * 512, (n + 1) * 512)
                pos = slice((n % SQC) * 512, (n % SQC + 1) * 512)
                if n not in hs_tiles:
                    hs_tiles[n] = hstp.tile([128, KT, 512], BF16, tag="hs", name=f"hs{n}")
                    for h in range(2):
                        nc.sync.dma_start(hs_tiles[n][:, h * 8:(h + 1) * 8, :],
                                          hsT_r[:, h * 8:(h + 1) * 8, tok])
                hs_t = hs_tiles[n]

                vt_sb = {}
                for m in range(HPC):
                    mh = slice(m * 128, (m + 1) * 128)
                    prj = {
                        p: p1ps.tile([128, 512], F32, tag=f"pj{p}", name=f"ps{p}{m}")
                        for p in "qkv"
                    }
                    for p in "qkv":
                        for k in range(KT):
                            nc.tensor.matmul(
                                prj[p], w_sb[p][:, k, mh], hs_t[:, k, :],
                                start=(k == 0),
                                stop=(k == KT - 1) and not with_bias,
                            )
                        if with_bias:
                            nc.tensor.matmul(
                                prj[p], b_sb[p][:, mh], ones_row.bitcast(F32R),
                                start=False, stop=True,
                            )

                    # RoPE on q/k: dst = ps*cos + rot(ps)*sinSw
                    for p, dst in (("q", qT), ("k", kTt)):
                        ps = prj[p]
                        t1 = ropet.tile([128, 512], BF16, tag="t1")
                        nc.vector.tensor_tensor(t1, ps, cos_sb[:, pos], op=ALU.mult)
                        t2 = ropet.tile([128, 512], BF16, tag="t2")
                        nc.vector.tensor_tensor(
                            t2[0:64], ps[64:128], sin_sb[64:128, pos], op=ALU.mult
                        )
                        nc.vector.tensor_tensor(
                            t2[64:128], ps[0:64], sin_sb[0:64, pos], op=ALU.mult
                        )
                        nc.vector.tensor_tensor(dst[m][:, tok], t1, t2, op=ALU.add)

                    # v: psum -> sbuf bf16, then DMA-XBAR transpose into
                    # the [S, d] layout (off the PE entirely)
                    vt_sb[m] = ropet.tile([128, 512], BF16, tag=f"vt{m}", name=f"vt{m}")
                    nc.vector.tensor_copy(vt_sb[m], prj["v"])
                    # dispatch on the (idle) scalar DGE queue so these
                    # don't head-of-line-block hs prefetch on the sync queue
                    nc.scalar.dma_start_transpose(
                        vN[m][:, n * 4:(n + 1) * 4, :], vt_sb[m]
                    )

        # ================= Phase 2: attention =================
        with (
            tc.tile_pool(name="epool", bufs=3) as epool,
            tc.tile_pool(name="opool", bufs=2) as opool,
            tc.tile_pool(name="stps", bufs=3, space="PSUM") as stps,
            tc.tile_pool(name="otps", bufs=1, space="PSUM") as otps,
            tc.tile_pool(name="dnps", bufs=1, space="PSUM") as dnps,
        ):
            def issue_scores(m, b, c):
                sq = slice(b * S + c * 512, b * S + (c + 1) * 512)
                e_all = epool.tile([128, SKT * 512], BF16, tag="e", name=f"e{m}{b}{c}")
                for pr in range(SKT // 2):
                    st_ps = stps.tile([128, 1024], F32, tag="st")
                    for j in range(2):
                        sk = 2 * pr + j
                        kblk = kTt[m][:, b * S + sk * 128: b * S + (sk + 1) * 128]
                        nc.tensor.matmul(
                            st_ps[:, j * 512:(j + 1) * 512],
                            kblk, qT[m][:, sq], start=True, stop=True,
                        )
                    nc.scalar.activation(
                        e_all[:, pr * 1024:(pr + 1) * 1024], st_ps,
                        AF.Exp, scale=ISCALE,
                    )
                return e_all

            def issue_consume(m, b, c, e_all):
                # PV accumulation
                ot_ps = otps.tile([128, 512], F32, tag="ot")
                for sk in range(SKT):
                    nc.tensor.matmul(
                        ot_ps, vN[m][:, b * SKT + sk, :],
                        e_all[:, sk * 512:(sk + 1) * 512],
                        start=(sk == 0), stop=(sk == SKT - 1),
                    )
                # denominator: 4 col-tiled ones-matmuls per pack run
                # concurrently on distinct 32-col PE groups
                dn_ps = dnps.tile([128, 512], F32, tag="dn")
                for p4 in range(4):
                    for g in range(4):
                        sk = 4 * g + p4
                        nc.tensor.matmul(
                            dn_ps[32 * g:32 * (g + 1), :], ones_c,
                            e_all[:, sk * 512:(sk + 1) * 512],
                            start=(p4 == 0), stop=(p4 == 3),
                            tile_position=(0, 32 * g),
                        )

                # psum -> sbuf (DVE), DMA-XBAR transposes, scale + DMA out
                ot_sb = opool.tile([128, 512], BF16, tag="ot_sb")
                nc.vector.tensor_copy(ot_sb, ot_ps)
                dn_sb = opool.tile([128, 512], BF16, tag="dn_sb")
                nc.vector.tensor_copy(dn_sb, dn_ps)
                otT = opool.tile([128, 4, 128], BF16, tag="otT")
                nc.sync.dma_start_transpose(otT, ot_sb)
                dnT = opool.tile([128, 4, 128], BF16, tag="dnT")
                nc.sync.dma_start_transpose(dnT, dn_sb)
                # transposed denominator partials sit at cols {0,32,64,96}
                sAB = opool.tile([128, 4, 2], F32, tag="sAB")
                nc.vector.tensor_tensor(
                    sAB, dnT[:, :, 0:64:32], dnT[:, :, 64:128:32], op=ALU.add
                )
                dsum = opool.tile([128, 4, 1], F32, tag="dsum")
                nc.vector.tensor_tensor(
                    dsum, sAB[:, :, 0:1], sAB[:, :, 1:2], op=ALU.add
                )
                rdt = opool.tile([128, 4, 1], F32, tag="rdt")
                nc.vector.reciprocal(rdt, dsum)
                o_sb = opool.tile([128, 4, 128], F32, tag="o")
                for blk in range(4):
                    nc.vector.tensor_scalar_mul(
                        o_sb[:, blk, :], otT[:, blk, :], rdt[:, blk, :]
                    )
                r0 = b * S + c * 512
                dst = out_d[r0:r0 + 512, m * 128:(m + 1) * 128].rearrange(
                    "(blk p) c -> p blk c", p=128
                )
                nc.sync.dma_start(dst, o_sb)

            # software pipeline: scores of group i issue before the consume
            # stage of group i-1, so ACT never starves at group boundaries
            groups = [(m, b, c) for m in range(HPC) for b in range(B)
                      for c in range(SQC)]
            prev = None
            for g in groups:
                e_cur = issue_scores(*g)
                if prev is not None:
                    issue_consume(*prev[0], prev[1])
                prev = (g, e_cur)
            issue_consume(*prev[0], prev[1])

    nc.compile()
    return nc


def _rope_tables():
    inv_freq = 1.0 / (ROPE_BASE ** (np.arange(0, D, 2, dtype=np.float64) / D))
    pos = np.arange(S, dtype=np.float64)
    ang = pos[:, None] * inv_freq[None, :]          # [S, D/2]
    emb = np.concatenate([ang, ang], axis=-1)       # [S, D]
    # only the 64 distinct rows ship; the kernel mirrors cos into rows
    # 64:128 and writes -sin there (t2[0:64] = q[64:128] * (-sin),
    # t2[64:128] = q[0:64] * (+sin))
    cosT = np.ascontiguousarray(np.cos(emb).T[0:64].astype(np.float32))
    sinT = np.ascontiguousarray(np.sin(emb).T[0:64].astype(np.float32))
    return cosT, sinT


def _ensure_axon_hooks():
    """bass_utils imports antenv.axon_hooks unconditionally when BASS_TRACE
    is set; this image's antenv package lacks that submodule. Synthesize it
    (and register the real NTFF hook when available) so tracing works and a
    bare import can't crash the run."""
    import sys
    import types

    try:
        import antenv.axon_hooks  # noqa: F401
        return
    except ImportError:
        pass
    try:
        import antenv
    except ImportError:
        return

    mod = types.ModuleType("antenv.axon_hooks")
    mod._hook = None
    mod.set_axon_ntff_profile_hook = lambda h: setattr(mod, "_hook", h)
    mod.get_axon_ntff_profile_hook = lambda: mod._hook
    sys.modules["antenv.axon_hooks"] = mod
    antenv.axon_hooks = mod
    try:
        from trn_agent_boot.trn_boot import _ntff_profile_via_ctypes

        mod._hook = _ntff_profile_via_ctypes("/opt/axon/libaxon_pjrt.so")
    except Exception:
        pass


def kernel(hidden_states, Wq, bq, Wk, bk, Wv, bv):
    global LAST_RESULT
    import ml_dtypes

    _ensure_axon_hooks()
    from concourse.bass_utils import run_bass_kernel_spmd

    BF = ml_dtypes.bfloat16
    hs = np.asarray(hidden_states, dtype=np.float32).reshape(BS, H)
    Wq = np.asarray(Wq, dtype=np.float32)
    Wk = np.asarray(Wk, dtype=np.float32)
    Wv = np.asarray(Wv, dtype=np.float32)
    bq = np.asarray(bq, dtype=np.float32)
    bk = np.asarray(bk, dtype=np.float32)
    bv = np.asarray(bv, dtype=np.float32)

    with_bias = bool(np.any(bq) or np.any(bk) or np.any(bv))
    nc = _build_nc(with_bias)

    hsT = np.ascontiguousarray(hs.T.astype(BF))     # [H, BS] bf16
    cosT, sinT = _rope_tables()

    in_maps = []
    for c in range(NCORES):
        ch = slice(c * CH, (c + 1) * CH)
        m = {
            "hsT": hsT,
            "wqT": np.ascontiguousarray(Wq[ch, :].T.astype(BF)),
            "wkT": np.ascontiguousarray(Wk[ch, :].T.astype(BF)),
            "wvT": np.ascontiguousarray(Wv[ch, :].T.astype(BF)),
            "cosT": cosT,
            "sinT": sinT,
        }
        if with_bias:
            m["bq"] = np.ascontiguousarray(bq[None, ch])
            m["bk"] = np.ascontiguousarray(bk[None, ch])
            m["bv"] = np.ascontiguousarray(bv[None, ch])
        else:
            z = np.zeros((1, CH), dtype=np.float32)
            m["bq"] = m["bk"] = m["bv"] = z
        in_maps.append(m)

    res = run_bass_kernel_spmd(nc, in_maps, core_ids=list(range(NCORES)))
    LAST_RESULT = res

    full = np.concatenate([r["out"] for r in res.results], axis=1)  # [BS, H]
    return full.reshape(B, S, H)


# revision 25
# speedup vs baseline: 1.0050x; 1.0050x over previous
"""Multi-head attention (QKV proj + RoPE + softmax attention) on 8 Trainium2
NeuronCores, tensor-parallel over heads (2 heads per core).

v2: bf16 operand pipeline (same PE matmul rate as f32r, half the DMA/SBUF,
2-4x DVE), head-ping-pong PSUM in the projection phase, 4-way column-tiled
denominator matmuls, 1024-wide exp straight from PSUM, Vector-engine copies.

Contract: kernel(**inputs) takes the FULL unsharded inputs and returns the
FULL [B, S, H] float32 output.
"""

from contextlib import ExitStack

import numpy as np

B, S, H = 2, 2048, 2048
NH, D = 16, 128
ROPE_BASE = 10000.0
NCORES = 8
HPC = NH // NCORES          # heads per core
CH = HPC * D                # output channels per core
BS = B * S                  # flattened tokens
KT = H // 128               # contraction k-tiles
NCH = BS // 512             # 512-wide token chunks
SKT = S // 128              # score k-tiles per sequence
SQC = S // 512              # query chunks per sequence

LAST_RESULT = None          # BassKernelResults of the most recent run (for test.py)


def _build_nc(with_bias):
    import concourse.mybir as mybir
    import concourse.tile as tile
    from concourse import bacc

    F32 = mybir.dt.float32
    F32R = mybir.dt.float32r
    BF16 = mybir.dt.bfloat16
    AF = mybir.ActivationFunctionType
    ALU = mybir.AluOpType
    ISCALE = float(1.0 / np.sqrt(D))

    nc = bacc.Bacc("TRN2", debug=False, enable_partition_id=False)

    hsT_d = nc.dram_tensor("hsT", [H, BS], BF16, kind="ExternalInput").ap()
    wT_d = {
        p: nc.dram_tensor(f"w{p}T", [H, CH], BF16, kind="ExternalInput").ap()
        for p in "qkv"
    }
    b_d = {
        p: nc.dram_tensor(f"b{p}", [1, CH], F32R, kind="ExternalInput").ap()
        for p in "qkv"
    }
    cos_d = nc.dram_tensor("cosT", [D // 2, S], F32, kind="ExternalInput").ap()
    sin_d = nc.dram_tensor("sinT", [D // 2, S], F32, kind="ExternalInput").ap()
    out_d = nc.dram_tensor("out", [BS, CH], F32, kind="ExternalOutput").ap()

    with tile.TileContext(nc) as tc, ExitStack() as ctx:
        # ---- persistent state (lives across both phases) ----
        persist = ctx.enter_context(tc.tile_pool(name="persist", bufs=1))
        qT = [persist.tile([128, BS], BF16, tag=f"qT{m}", name=f"qT{m}") for m in range(HPC)]
        kTt = [persist.tile([128, BS], BF16, tag=f"kT{m}", name=f"kT{m}") for m in range(HPC)]
        vN = [persist.tile([128, BS // 128, D], BF16, tag=f"v{m}", name=f"vn{m}") for m in range(HPC)]

        consts = ctx.enter_context(tc.tile_pool(name="consts", bufs=1))
        ones_c = consts.tile([128, 32], BF16, tag="ones_c")
        nc.vector.memset(ones_c, 1.0)
        if with_bias:
            ones_row = consts.tile([1, 512], F32, tag="ones_row")
            nc.vector.memset(ones_row, 1.0)
            b_sb = {}
            for p in "qkv":
                b_sb[p] = consts.tile([1, CH], F32R, tag=f"b{p}", name=f"b{p}sb")
                nc.sync.dma_start(b_sb[p], b_d[p])

        # ================= Phase 1: QKV projections + RoPE =================
        with (
            tc.tile_pool(name="wpool", bufs=1) as wpool,
            tc.tile_pool(name="tabs", bufs=1) as tabs,
            tc.tile_pool(name="hstp", bufs=4) as hstp,
            tc.tile_pool(name="p1ps", bufs=2, space="PSUM") as p1ps,
            tc.tile_pool(name="ropet", bufs=3) as ropet,
        ):
            w_sb = {}
            for p in "qkv":
                w_sb[p] = wpool.tile([128, KT, CH], BF16, tag=f"w{p}", name=f"w{p}sb")
            w_r = {p: wT_d[p].rearrange("(k p) c -> p k c", p=128) for p in "qkv"}
            hsT_r = hsT_d.rearrange("(k p) t -> p k t", p=128)
            cos_sb = tabs.tile([D, S], F32, tag="cos")
            sin_sb = tabs.tile([D, S], F32, tag="sin")

            # DMA issue order tuned for a fast start: wq halves, first hs
            # chunk, then the rest.
            hs_tiles = {}
            hs_tiles[0] = hstp.tile([128, KT, 512], BF16, tag="hs", name="hs0")
            for lo, hi in ((0, 1), (1, 2), (2, 4), (4, 8), (8, 12), (12, 16)):
                hh = slice(lo, hi)
                nc.sync.dma_start(w_sb["q"][:, hh, :], w_r["q"][:, hh, :])
                nc.sync.dma_start(hs_tiles[0][:, hh, :], hsT_r[:, hh, 0:512])
            for p in "kv":
                for h in range(2):
                    nc.sync.dma_start(w_sb[p][:, h * 8:(h + 1) * 8, :],
                                      w_r[p][:, h * 8:(h + 1) * 8, :])
            # rope tables have duplicated 64-row halves; ship only the
            # distinct rows (1MB not 2MB in the DMA-bound front, scalar
            # queue), mirror/negate on-chip once on the idle DVE.
            nc.scalar.dma_start(cos_sb[0:64, :], cos_d)
            nc.scalar.dma_start(sin_sb[0:64, :], sin_d)
            nc.vector.tensor_copy(cos_sb[64:128, :], cos_sb[0:64, :])
            nc.vector.tensor_scalar_mul(sin_sb[64:128, :], sin_sb[0:64, :], -1.0)

            for n in range(NCH):
                tok = slice(n * 512, (n + 1) * 512)
                pos = slice((n % SQC) * 512, (n % SQC + 1) * 512)
                if n not in hs_tiles:
                    hs_tiles[n] = hstp.tile([128, KT, 512], BF16, tag="hs", name=f"hs{n}")
                    for h in range(2):
                        nc.sync.dma_start(hs_tiles[n][:, h * 8:(h + 1) * 8, :],
                                          hsT_r[:, h * 8:(h + 1) * 8, tok])
                hs_t = hs_tiles[n]

                vt_sb = {}
                for m in range(HPC):
                    mh = slice(m * 128, (m + 1) * 128)
                    prj = {
                        p: p1ps.tile([128, 512], F32, tag=f"pj{p}", name=f"ps{p}{m}")
                        for p in "qkv"
                    }
                    for p in "qkv":
                        for k in range(KT):
                            nc.tensor.matmul(
                                prj[p], w_sb[p][:, k, mh], hs_t[:, k, :],
                                start=(k == 0),
                                stop=(k == KT - 1) and not with_bias,
                            )
                        if with_bias:
                            nc.tensor.matmul(
                                prj[p], b_sb[p][:, mh], ones_row.bitcast(F32R),
                                start=False, stop=True,
                            )

                    # RoPE on q/k: dst = ps*cos + rot(ps)*sinSw
                    for p, dst in (("q", qT), ("k", kTt)):
                        ps = prj[p]
                        t1 = ropet.tile([128, 512], BF16, tag="t1")
                        nc.vector.tensor_tensor(t1, ps, cos_sb[:, pos], op=ALU.mult)
                        t2 = ropet.tile([128, 512], BF16, tag="t2")
                        nc.vector.tensor_tensor(
                            t2[0:64], ps[64:128], sin_sb[64:128, pos], op=ALU.mult
                        )
                        nc.vector.tensor_tensor(
                            t2[64:128], ps[0:64], sin_sb[0:64, pos], op=ALU.mult
                        )
                        nc.vector.tensor_tensor(dst[m][:, tok], t1, t2, op=ALU.add)

                    # v: psum -> sbuf bf16, then DMA-XBAR transpose into
                    # the [S, d] layout (off the PE entirely)
                    vt_sb[m] = ropet.tile([128, 512], BF16, tag=f"vt{m}", name=f"vt{m}")
                    nc.vector.tensor_copy(vt_sb[m], prj["v"])
                    # dispatch on the (idle) scalar DGE queue so these
                    # don't head-of-line-block hs prefetch on the sync queue
                    nc.scalar.dma_start_transpose(
                        vN[m][:, n * 4:(n + 1) * 4, :], vt_sb[m]
                    )

        # ================= Phase 2: attention =================
        with (
            tc.tile_pool(name="epool", bufs=3) as epool,
            tc.tile_pool(name="opool", bufs=2) as opool,
            tc.tile_pool(name="stps", bufs=3, space="PSUM") as stps,
            tc.tile_pool(name="otps", bufs=1, space="PSUM") as otps,
            tc.tile_pool(name="dnps", bufs=1, space="PSUM") as dnps,
        ):
            def issue_scores(m, b, c):
                sq = slice(b * S + c * 512, b * S + (c + 1) * 512)
                e_all = epool.tile([128, SKT * 512], BF16, tag="e", name=f"e{m}{b}{c}")
                for pr in range(SKT // 2):
                    st_ps = stps.tile([128, 1024], F32, tag="st")
                    for j in range(2):
                        sk = 2 * pr + j
                        kblk = kTt[m][:, b * S + sk * 128: b * S + (sk + 1) * 128]
                        nc.tensor.matmul(
                            st_ps[:, j * 512:(j + 1) * 512],
                            kblk, qT[m][:, sq], start=True, stop=True,
                        )
                    nc.scalar.activation(
                        e_all[:, pr * 1024:(pr + 1) * 1024], st_ps,
                        AF.Exp, scale=ISCALE,
                    )
                return e_all

            def issue_consume(m, b, c, e_all):
                # PV accumulation
                ot_ps = otps.tile([128, 512], F32, tag="ot")
                for sk in range(SKT):
                    nc.tensor.matmul(
                        ot_ps, vN[m][:, b * SKT + sk, :],
                        e_all[:, sk * 512:(sk + 1) * 512],
                        start=(sk == 0), stop=(sk == SKT - 1),
                    )
                # denominator: 4 col-tiled ones-matmuls per pack run
                # concurrently on distinct 32-col PE groups
                dn_ps = dnps.tile([128, 512], F32, tag="dn")
                for p4 in range(4):
                    for g in range(4):
                        sk = 4 * g + p4
                        nc.tensor.matmul(
                            dn_ps[32 * g:32 * (g + 1), :], ones_c,
                            e_all[:, sk * 512:(sk + 1) * 512],
                            start=(p4 == 0), stop=(p4 == 3),
                            tile_position=(0, 32 * g),
                        )

                # psum -> sbuf (DVE), DMA-XBAR transposes, scale + DMA out
                ot_sb = opool.tile([128, 512], BF16, tag="ot_sb")
                nc.vector.tensor_copy(ot_sb, ot_ps)
                dn_sb = opool.tile([128, 512], BF16, tag="dn_sb")
                nc.vector.tensor_copy(dn_sb, dn_ps)
                otT = opool.tile([128, 4, 128], BF16, tag="otT")
                nc.sync.dma_start_transpose(otT, ot_sb)
                dnT = opool.tile([128, 4, 128], BF16, tag="dnT")
                nc.sync.dma_start_transpose(dnT, dn_sb)
                # transposed denominator partials sit at cols {0,32,64,96}
                sAB = opool.tile([128, 4, 2], F32, tag="sAB")
                nc.vector.tensor_tensor(
                    sAB, dnT[:, :, 0:64:32], dnT[:, :, 64:128:32], op=ALU.add
                )
                dsum = opool.tile([128, 4, 1], F32, tag="dsum")
                nc.vector.tensor_tensor(
                    dsum, sAB[:, :, 0:1], sAB[:, :, 1:2], op=ALU.add
                )
                rdt = opool.tile([128, 4, 1], F32, tag="rdt")
                nc.vector.reciprocal(rdt, dsum)
                o_sb = opool.tile([128, 4, 128], F32, tag="o")
                for blk in range(4):
                    nc.vector.tensor_scalar_mul(
                        o_sb[:, blk, :], otT[:, blk, :], rdt[:, blk, :]
                    )
                r0 = b * S + c * 512
                dst = out_d[r0:r0 + 512, m * 128:(m + 1) * 128].rearrange(
                    "(blk p) c -> p blk c", p=128
                )
                nc.sync.dma_start(dst, o_sb)

            # software pipeline: scores of group i issue before the consume
            # stage of group i-1, so ACT never starves at group boundaries
            groups = [(m, b, c) for m in range(HPC) for b in range(B)
                      for c in range(SQC)]
            prev = None
            for g in groups:
                e_cur = issue_scores(*g)
                if prev is not None:
                    issue_consume(*prev[0], prev[1])
                prev = (g, e_cur)
            issue_consume(*prev[0], prev[1])

    nc.compile()
    return nc


def _rope_tables():
    inv_freq = 1.0 / (ROPE_BASE ** (np.arange(0, D, 2, dtype=np.float64) / D))
    pos = np.arange(S, dtype=np.float64)
    ang = pos[:, None] * inv_freq[None, :]          # [S, D/2]
    emb = np.concatenate([ang, ang], axis=-1)       # [S, D]
    # only the 64 distinct rows ship; the kernel mirrors cos into rows
    # 64:128 and writes -sin there (t2[0:64] = q[64:128] * (-sin),
    # t2[64:128] = q[0:64] * (+sin))
    cosT = np.ascontiguousarray(np.cos(emb).T[0:64].astype(np.float32))
    sinT = np.ascontiguousarray(np.sin(emb).T[0:64].astype(np.float32))
    return cosT, sinT


def _ensure_axon_hooks():
    """bass_utils imports antenv.axon_hooks unconditionally when BASS_TRACE
    is set; this image's antenv package lacks that submodule. Synthesize it
    (and register the real NTFF hook when available) so tracing works and a
    bare import can't crash the run."""
    import sys
    import types

    try:
        import antenv.axon_hooks  # noqa: F401
        return
    except ImportError:
        pass
    try:
        import antenv
    except ImportError:
        return

    mod = types.ModuleType("antenv.axon_hooks")
    mod._hook = None
    mod.set_axon_ntff_profile_hook = lambda h: setattr(mod, "_hook", h)
    mod.get_axon_ntff_profile_hook = lambda: mod._hook
    sys.modules["antenv.axon_hooks"] = mod
    antenv.axon_hooks = mod
    try:
        from trn_agent_boot.trn_boot import _ntff_profile_via_ctypes

        mod._hook = _ntff_profile_via_ctypes("/opt/axon/libaxon_pjrt.so")
    except Exception:
        pass


def kernel(hidden_states, Wq, bq, Wk, bk, Wv, bv):
    global LAST_RESULT
    import ml_dtypes

    _ensure_axon_hooks()
    from concourse.bass_utils import run_bass_kernel_spmd

    BF = ml_dtypes.bfloat16
    hs = np.asarray(hidden_states, dtype=np.float32).reshape(BS, H)
    Wq = np.asarray(Wq, dtype=np.float32)
    Wk = np.asarray(Wk, dtype=np.float32)
    Wv = np.asarray(Wv, dtype=np.float32)
    bq = np.asarray(bq, dtype=np.float32)
    bk = np.asarray(bk, dtype=np.float32)
    bv = np.asarray(bv, dtype=np.float32)

    with_bias = bool(np.any(bq) or np.any(bk) or np.any(bv))
    nc = _build_nc(with_bias)

    hsT = np.ascontiguousarray(hs.T.astype(BF))     # [H, BS] bf16
    cosT, sinT = _rope_tables()

    in_maps = []
    for c in range(NCORES):
        ch = slice(c * CH, (c + 1) * CH)
        m = {
            "hsT": hsT,
            "wqT": np.ascontiguousarray(Wq[ch, :].T.astype(BF)),
            "wkT": np.ascontiguousarray(Wk[ch, :].T.astype(BF)),
            "wvT": np.ascontiguousarray(Wv[ch, :].T.astype(BF)),
            "cosT": cosT,
            "sinT": sinT,
        }
        if with_bias:
            m["bq"] = np.ascontiguousarray(bq[None, ch])
            m["bk"] = np.ascontiguousarray(bk[None, ch])
            m["bv"] = np.ascontiguousarray(bv[None, ch])
        else:
            z = np.zeros((1, CH), dtype=np.float32)
            m["bq"] = m["bk"] = m["bv"] = z
        in_maps.append(m)

    res = run_bass_kernel_spmd(nc, in_maps, core_ids=list(range(NCORES)))
    LAST_RESULT = res

    full = np.concatenate([r["out"] for r in res.results], axis=1)  # [BS, H]
    return full.reshape(B, S, H)
